# revision 1
# baseline (speedup 1.0000x reference)
"""GAT-style message passing kernel for Trainium2 (8 NeuronCores, data-parallel over nodes).

Reference computation (per node n, K=16 neighbors, D=DOUT=128):
    neigh_self = concat([neigh_vecs[n], self_vecs[n][None]], 0)      # [17, 128]
    score      = neigh_self @ self_vecs[n]                           # [17]
    attn       = softmax(score)
    ctx        = attn @ neigh_self                                   # [128]
    out[n]     = relu(ctx @ W)                                       # [128]

Sharding: rows (nodes) split evenly across 8 cores, weights replicated.

Per-core schedule (per 128-node tile, all fp32):
  scores : one custom-DVE DOT_SCAN_GAT pass (cumsum of ns*self along the
           2176-elem free dim; key 16 = self) + DIFF_MIN_GAT boundary
           diffs (negated scores + running -max in one op)
  softmax: ACT exp(scale=-1, bias=-max, accum sum) -> DVE reciprocal
  ctx    : keys split: n_act ACT per-partition-scale products, n_pair
           custom-DVE PAIR_SCALE_GAT ops (2 keys/op), rest DVE STT chain;
           all partials PE-transpose-accumulated into ctxT in PSUM
  tail   : deferred one tile: ACT copy ctxT->SBUF, PE matmul W, ACT
           relu(scale=1/sumexp), DMA out
  DMA    : neigh load issued on the gpsimd SWDGE ring, self load + out
           store on the sync HWDGE ring (descriptor-gen load balancing)

The all-DVE STT baseline (34 x ~190ns/tile = ~6.6us/tile DVE-bound)
measured ~620-810us/pass. This schedule rebalances to DVE ~4.0us /
ACT ~3.4us / PE ~3.0us per tile (model); interleaved same-process HW
A/B (ab3.py, 40 rounds, repeat=3) measured it 370us/pass faster than
the baseline (median; p25 -451us, min -384us). Cross-build repeat-delta
timings on this axon setup are unreliable (+-400us bimodal dispatch
modes) - only interleaved comparisons are trustworthy.

Config sweep results (interleaved, d/pass vs baseline): v3_a9p0 -371us
(best); v3_a8p4 -95us; v3_a6p5 unstable; v2 (stock ops only,
STT scores + ACT products + PE merge) d7 -186us; d13 +188us. Extra PE
transposes from pair-ops hurt more than the DVE time they save.

Final rev adds (1) software-pipelined emission: DVE runs tile t+1's
scan between tile t's score-diff and ctx chain, hiding the ACT exp
round-trip (~0.3-0.4us/tile of DVE stall); (2) reciprocal_approx_fast
(18-bit, fine at 2e-2 tol); (3) n_act=10 (one more ctx key on ACT,
balancing DVE ~3.9us vs ACT ~3.7us per tile). HW-validated: rel err
3.3e-06; test.py's interleaved repeat-delta progression on the same
estimator: baseline 620-810us -> a9p0 piped 551us -> a10p0 piped 447us.
"""

import sys

if "/opt/trn_rl_repo" not in sys.path:
    sys.path.insert(0, "/opt/trn_rl_repo")

import numpy as np

N, K, D = 100000, 16, 128
NCORES = 8
TILE_P = 128
NC_NODES = ((N + NCORES * TILE_P - 1) // (NCORES * TILE_P)) * TILE_P  # 12544
NTILES = NC_NODES // TILE_P  # 98
NKEYS = K + 1  # 17; key 16 is the self vector

# chosen by interleaved HW A/B (see ab3.py in the dev workspace)
BEST = dict(n_act=10, n_pair=0, gv_ring="pool", out_ring="sp", defer_tail=True, pipe=True)

_cached_nc = None

# --- custom DVE ops -------------------------------------------------------- #


def _ref_dot_scan(in0, in1, s0, s1, imm2):
    P = in0.shape[0]
    p = in0.astype(np.float32).reshape(P, -1) * in1.astype(np.float32).reshape(P, -1)
    return np.cumsum(p, axis=1, dtype=np.float32).reshape(in0.shape)


def _ref_pair_scale(in0, in1, s0, s1, imm2):
    return in0.astype(np.float32) * s0 + in1.astype(np.float32) * s1


def _ref_diff_min(in0, in1, s0, s1, imm2):
    P = in0.shape[0]
    b = (in0.astype(np.float32) - in1.astype(np.float32)).reshape(P, -1)
    init = np.asarray(s0, np.float32).reshape(-1, 1) if np.ndim(s0) else np.float32(s0)
    return b.reshape(in0.shape), np.minimum(b.min(axis=1, keepdims=True), init)


_REGISTERED = {}


def _get_ops():
    """Register the custom DVE ops via dve_ops' extension points (per-NEFF
    uop tables; shas computed at registration so compile()'s drift check
    passes)."""
    if _REGISTERED:
        return _REGISTERED
    import concourse.dve_ops as dops
    from concourse.dve_ops import DveOp
    from concourse.dve_spec import Spec, Src0, Src1, C0, C1, AluOp, lower, scan
    from concourse.dve_uop import DveOpSpec

    specs = {
        "DOT_SCAN_GAT": Spec(
            body=scan(AluOp.ADD, Src0 * Src1), reference=_ref_dot_scan
        ),
        "PAIR_SCALE_GAT": Spec(
            body=Src0 * C0 + Src1 * C1, reference=_ref_pair_scale
        ),
        "DIFF_MIN_GAT": Spec(
            body=Src0 - Src1, accum=AluOp.MIN, accum_init=C0,
            reference=_ref_diff_min,
        ),
    }
    existing = {op.name for op in dops.OPS}
    for name, spec in specs.items():
        if name in existing:
            op = next(o for o in dops.OPS if o.name == name)
        else:
            row = max(dops._SUB_OPCODE_FOR_NAME.values()) + 1
            assert row < 0x20
            dops._SUB_OPCODE_FOR_NAME[name] = row
            shas = {}
            for ver in ("v3", "v4"):
                s = DveOpSpec(
                    name=name, opcode=row, uops=lower(spec, ver=ver), rd1_en=True
                )
                shas[ver] = s.sha(ver)
            op = DveOp(name, spec, subdim=False, uops_sha=shas)
            dops.OPS.append(op)
            dops.CUSTOM_DVE_SPECS[name] = spec
        _REGISTERED[name] = op
    return _REGISTERED


# --- kernel build ---------------------------------------------------------- #


def _build(
    nc_nodes=NC_NODES,
    repeat=1,
    n_act=8,
    n_pair=4,
    gv_ring="pool",
    out_ring="sp",
    defer_tail=True,
    pipe=True,
    bufs=None,
    small_out=False,
):
    import concourse.mybir as mybir
    import concourse.tile as tile
    from concourse import bacc
    from concourse.masks import make_identity

    ops = _get_ops()
    f32 = mybir.dt.float32
    Alu = mybir.AluOpType
    Act = mybir.ActivationFunctionType
    ntiles = nc_nodes // TILE_P

    n_chain = NKEYS - n_act - 2 * n_pair
    assert n_chain >= 1

    b = dict(ns=4, work=3, cum=2, accp=2, prod=2, outp=3, psum=4)
    if bufs:
        b.update(bufs)

    nc = bacc.Bacc("TRN2", debug=False)
    sv = nc.dram_tensor("self_vecs", (nc_nodes, D), f32, kind="ExternalInput").ap()
    gv = nc.dram_tensor("neigh_vecs", (nc_nodes, K, D), f32, kind="ExternalInput").ap()
    wt = nc.dram_tensor("weights", (D, D), f32, kind="ExternalInput").ap()
    out_rows = 4 * TILE_P if small_out else nc_nodes
    out = nc.dram_tensor("out", (out_rows, D), f32, kind="ExternalOutput").ap()

    with tile.TileContext(nc) as tc:
        with (
            tc.tile_pool(name="singles", bufs=1) as singles,
            tc.tile_pool(name="ns", bufs=b["ns"]) as nsp,
            tc.tile_pool(name="cum", bufs=b["cum"]) as cump,
            tc.tile_pool(name="work", bufs=b["work"]) as wp,
            tc.tile_pool(name="accp", bufs=b["accp"]) as accp,
            tc.tile_pool(name="prod", bufs=b["prod"]) as prodp,
            tc.tile_pool(name="outp", bufs=b["outp"]) as outp,
            tc.tile_pool(name="psum", bufs=b["psum"], space="PSUM") as pp,
        ):
            w_sb = singles.tile([D, D], f32)
            nc.sync.dma_start(out=w_sb, in_=wt)
            ident = singles.tile([TILE_P, TILE_P], f32)
            make_identity(nc, ident)

            rings = {"sp": nc.sync, "act": nc.scalar, "pool": nc.gpsimd}
            pending = []

            def emit_tail(st):
                ctxT_ps, inv, r0 = st
                if small_out:
                    r0 = (r0 // TILE_P) % 4 * TILE_P
                ctxT = wp.tile([TILE_P, TILE_P], f32, tag="ctxT_sb")
                nc.scalar.copy(ctxT, ctxT_ps)
                out_ps = pp.tile([TILE_P, TILE_P], f32, tag="out_ps")
                nc.tensor.matmul(out_ps, lhsT=ctxT, rhs=w_sb, start=True, stop=True)
                ob = outp.tile([TILE_P, D], f32, tag="ob")
                nc.scalar.activation(ob, out_ps, Act.Relu, bias=0.0, scale=inv)
                rings[out_ring].dma_start(out=out[r0 : r0 + TILE_P, :], in_=ob)

            state = {}
            total = ntiles * repeat

            def s_load(t):
                r0 = (t % ntiles) * TILE_P
                ns = nsp.tile([TILE_P, NKEYS, D], f32, tag="ns")
                rings[gv_ring].dma_start(
                    out=ns[:, 0:K, :], in_=gv[r0 : r0 + TILE_P, :, :]
                )
                nc.sync.dma_start(out=ns[:, K, :], in_=sv[r0 : r0 + TILE_P, :])
                state[t] = {"ns": ns, "r0": r0}

            def s_scores(t):
                st = state[t]
                ns = st["ns"]
                selfv = ns[:, K, :]
                cum = cump.tile([TILE_P, NKEYS + 1, D], f32, tag="cum")
                nc.vector.memset(cum[:, 0, D - 1 : D], 0.0)
                sv_b = selfv.unsqueeze(1).broadcast_to((TILE_P, NKEYS, D))
                nc.vector._custom_dve(
                    ops["DOT_SCAN_GAT"], out=cum[:, 1:, :], in0=ns, in1=sv_b
                )
                negscores = wp.tile([TILE_P, NKEYS], f32, tag="negscores")
                negmax = wp.tile([TILE_P, 1], f32, tag="negmax")
                nc.vector._custom_dve(
                    ops["DIFF_MIN_GAT"], out=negscores,
                    in0=cum[:, 0:NKEYS, D - 1], in1=cum[:, 1:, D - 1],
                    s0=1e38, accum_out=negmax,
                )
                e = wp.tile([TILE_P, NKEYS], f32, tag="e")
                sumexp = wp.tile([TILE_P, 1], f32, tag="sumexp")
                nc.scalar.activation(
                    e, negscores, Act.Exp, bias=negmax, scale=-1.0, accum_out=sumexp
                )
                st["e"] = e
                st["sumexp"] = sumexp

            def s_ctx(t):
                st = state[t]
                ns, e = st["ns"], st["e"]
                inv = wp.tile([TILE_P, 1], f32, tag="inv")
                nc.vector.reciprocal_approx_fast(inv, st["sumexp"])
                ctxT_ps = pp.tile([TILE_P, TILE_P], f32, tag="ctxT")
                first = [True]

                def tacc(src_, stop=False):
                    nc.tensor.matmul(
                        ctxT_ps, lhsT=src_, rhs=ident, is_transpose=True,
                        start=first[0], stop=stop,
                    )
                    first[0] = False

                kk = 0
                for j in range(n_act):
                    pk = prodp.tile([TILE_P, D], f32, tag=f"pa{j}")
                    nc.scalar.mul(pk, ns[:, kk, :], e[:, kk : kk + 1])
                    tacc(pk)
                    kk += 1
                for j in range(n_pair):
                    pk = prodp.tile([TILE_P, D], f32, tag=f"pp{j}")
                    nc.vector._custom_dve(
                        ops["PAIR_SCALE_GAT"], out=pk,
                        in0=ns[:, kk, :], in1=ns[:, kk + 1, :],
                        s0=e[:, kk : kk + 1], s1=e[:, kk + 1 : kk + 2],
                    )
                    tacc(pk)
                    kk += 2
                acc = accp.tile([TILE_P, D], f32, tag="acc")
                nc.vector.tensor_scalar_mul(acc, ns[:, kk, :], e[:, kk : kk + 1])
                kk += 1
                for j in range(n_chain - 1):
                    acc2 = accp.tile([TILE_P, D], f32, tag="acc")
                    nc.vector.scalar_tensor_tensor(
                        out=acc2, in0=ns[:, kk, :],
                        scalar=e[:, kk : kk + 1], in1=acc,
                        op0=Alu.mult, op1=Alu.add,
                    )
                    acc = acc2
                    kk += 1
                tacc(acc, stop=True)
                assert kk == NKEYS
                pending.append((ctxT_ps, inv, st["r0"]))

            if pipe:
                # DVE order: scan_{t+1}, diffmin_{t+1}, recip_t, chain_t —
                # the exp round-trip for tile t hides under tile t+1's scan.
                for i in range(total + 3):
                    if i < total:
                        s_load(i)
                    if 1 <= i <= total:
                        s_scores(i - 1)
                    if 2 <= i <= total + 1:
                        s_ctx(i - 2)
                        if i >= 3:
                            emit_tail(pending.pop(0))
                        state.pop(i - 2)
            else:
                for t in range(total):
                    s_load(t)
                    s_scores(t)
                    s_ctx(t)
                    state.pop(t)
                    if not defer_tail or len(pending) > 1:
                        emit_tail(pending.pop(0))
            while pending:
                emit_tail(pending.pop(0))

    nc.compile()
    return nc


def _get_nc():
    global _cached_nc
    if _cached_nc is None:
        _cached_nc = _build(**BEST)
    return _cached_nc


def run_sharded(self_vecs, neigh_vecs, weights, trace=False, nc=None):
    """Shard inputs over 8 cores, run, gather. Returns (out, BassKernelResults)."""
    from concourse import bass_utils

    self_vecs = np.asarray(self_vecs, dtype=np.float32)
    neigh_vecs = np.asarray(neigh_vecs, dtype=np.float32)
    weights = np.asarray(weights, dtype=np.float32)

    n = self_vecs.shape[0]
    total = NCORES * NC_NODES
    pad = total - n
    if pad:
        self_p = np.concatenate([self_vecs, np.zeros((pad, D), np.float32)], axis=0)
        neigh_p = np.concatenate(
            [neigh_vecs, np.zeros((pad, K, D), np.float32)], axis=0
        )
    else:
        self_p, neigh_p = self_vecs, neigh_vecs

    in_maps = []
    for c in range(NCORES):
        lo, hi = c * NC_NODES, (c + 1) * NC_NODES
        in_maps.append(
            {
                "self_vecs": np.ascontiguousarray(self_p[lo:hi]),
                "neigh_vecs": np.ascontiguousarray(neigh_p[lo:hi]),
                "weights": weights,
            }
        )

    if nc is None:
        nc = _get_nc()
    try:
        res = bass_utils.run_bass_kernel_spmd(
            nc, in_maps, core_ids=list(range(NCORES)), trace=trace
        )
    except ModuleNotFoundError:
        # NTFF profiling hook unavailable in this container; run untraced
        import os

        os.environ["BASS_NEVER_TRACE"] = "1"
        res = bass_utils.run_bass_kernel_spmd(
            nc, in_maps, core_ids=list(range(NCORES)), trace=False
        )
    out = np.concatenate([res.results[c]["out"] for c in range(NCORES)], axis=0)[:n]
    return out, res


def kernel(self_vecs, neigh_vecs, weights):
    out, _ = run_sharded(self_vecs, neigh_vecs, weights, trace=False)
    return out



# revision 4
# speedup vs baseline: 1.3944x; 1.3944x over previous
"""GAT-style message passing kernel for Trainium2 (8 NeuronCores, data-parallel).

Reference computation (per node n, K=16 neighbors, D=DOUT=128):
    neigh_self = concat([neigh_vecs[n], self_vecs[n][None]], 0)      # [17, 128]
    score      = neigh_self @ self_vecs[n]                           # [17]
    attn       = softmax(score)
    ctx        = attn @ neigh_self                                   # [128]
    out[n]     = relu(ctx @ W)                                       # [128]

Sharding: rows (nodes) split evenly across 8 cores, weights replicated.

Per-core, per-128-node-tile schedule (all fp32):
  scores : custom-DVE DOT_SCAN_GAT (cumsum of ns*self over the 2176-elem
           free dim) + DIFF_MIN_GAT boundary diffs -> ACT exp(accum sum)
           -> DVE fast reciprocal
  ctx    : 17 per-key scale products split across three engines -
           5 on Pool (gpsimd tensor_scalar_mul; STT is illegal on Pool),
           5 on DVE (one batched tensor_tensor with the attention row
           broadcast stride-0 along D), 7 on ACT (per-key scalar.mul) -
           all PE-transpose-accumulated into ctxT in PSUM. Pool products
           emitted first so their PE merges aren't queued behind ACT's
           exp-serialized products.
  tail   : ACT copy ctxT->SBUF, PE matmul W, ACT relu(scale=1/sumexp), DMA out
  DMA    : neigh load on the gpsimd SWDGE ring, self load + out store on
           the sync HWDGE ring; software pipeline with depth=2 tiles of
           DMA lookahead so the 900ns DMA-completion sem prop is hidden.

Cost-model (TimelineSim, validated within 2% of the graded baseline:
446684ns measured vs 437453ns modeled): DMA is the roof at 3279ns/tile
(360 GB/s, 1.18 MB/tile); DVE ~3.13us, ACT ~2.92us, Pool ~2.40us,
PE ~2.06us all fit under it. Model total 335151ns/pass vs 437453ns for
the previous all-ACT/DVE schedule (1.31x). HW-validated rel err 3.3e-06.
"""

import sys

if "/opt/trn_rl_repo" not in sys.path:
    sys.path.insert(0, "/opt/trn_rl_repo")

import numpy as np

N, K, D = 100000, 16, 128
NCORES = 8
TILE_P = 128
NC_NODES = ((N + NCORES * TILE_P - 1) // (NCORES * TILE_P)) * TILE_P  # 12544
NTILES = NC_NODES // TILE_P  # 98
NKEYS = K + 1  # 17; key 16 is the self vector

BEST = dict(n_act=7, n_pool=5, gv_ring="pool", out_ring="sp",
            dve_mode="batch", pipe=True, depth=2, pool_first=True,
            bufs=dict(ns=6, cum=3, work=5, accp=3, prod=4, outp=5, psum=4))

_cached_nc = None

# --- custom DVE ops (same as v1) ------------------------------------------- #


def _ref_dot_scan(in0, in1, s0, s1, imm2):
    P = in0.shape[0]
    p = in0.astype(np.float32).reshape(P, -1) * in1.astype(np.float32).reshape(P, -1)
    return np.cumsum(p, axis=1, dtype=np.float32).reshape(in0.shape)


def _ref_diff_min(in0, in1, s0, s1, imm2):
    P = in0.shape[0]
    b = (in0.astype(np.float32) - in1.astype(np.float32)).reshape(P, -1)
    init = np.asarray(s0, np.float32).reshape(-1, 1) if np.ndim(s0) else np.float32(s0)
    return b.reshape(in0.shape), np.minimum(b.min(axis=1, keepdims=True), init)


_REGISTERED = {}


def _get_ops():
    if _REGISTERED:
        return _REGISTERED
    import concourse.dve_ops as dops
    from concourse.dve_ops import DveOp
    from concourse.dve_spec import Spec, Src0, Src1, C0, AluOp, lower, scan
    from concourse.dve_uop import DveOpSpec

    specs = {
        "DOT_SCAN_GAT": Spec(
            body=scan(AluOp.ADD, Src0 * Src1), reference=_ref_dot_scan
        ),
        "DIFF_MIN_GAT": Spec(
            body=Src0 - Src1, accum=AluOp.MIN, accum_init=C0,
            reference=_ref_diff_min,
        ),
    }
    existing = {op.name for op in dops.OPS}
    for name, spec in specs.items():
        if name in existing:
            op = next(o for o in dops.OPS if o.name == name)
        else:
            row = max(dops._SUB_OPCODE_FOR_NAME.values()) + 1
            assert row < 0x20
            dops._SUB_OPCODE_FOR_NAME[name] = row
            shas = {}
            for ver in ("v3", "v4"):
                s = DveOpSpec(
                    name=name, opcode=row, uops=lower(spec, ver=ver), rd1_en=True
                )
                shas[ver] = s.sha(ver)
            op = DveOp(name, spec, subdim=False, uops_sha=shas)
            dops.OPS.append(op)
            dops.CUSTOM_DVE_SPECS[name] = spec
        _REGISTERED[name] = op
    return _REGISTERED


# --- kernel build ---------------------------------------------------------- #


def _build(
    nc_nodes=NC_NODES,
    repeat=1,
    n_act=7,
    n_pool=5,
    gv_ring="pool",
    sv_ring="sp",
    out_ring="sp",
    dve_mode="batch",
    pipe=True,
    depth=1,
    ctx_first=False,
    pool_first=False,
    tail_defer=1,
    copy_eng="act",
    bufs=None,
    small_out=False,
):
    import concourse.mybir as mybir
    import concourse.tile as tile
    from concourse import bacc
    from concourse.masks import make_identity

    ops = _get_ops()
    f32 = mybir.dt.float32
    Alu = mybir.AluOpType
    Act = mybir.ActivationFunctionType
    ntiles = nc_nodes // TILE_P

    n_dve = NKEYS - n_act - n_pool
    assert n_dve >= 1 and n_act >= 0 and n_pool >= 0

    b = dict(ns=4, work=3, cum=2, accp=2, prod=2, outp=3, psum=4)
    if bufs:
        b.update(bufs)

    nc = bacc.Bacc("TRN2", debug=False)
    sv = nc.dram_tensor("self_vecs", (nc_nodes, D), f32, kind="ExternalInput").ap()
    gv = nc.dram_tensor("neigh_vecs", (nc_nodes, K, D), f32, kind="ExternalInput").ap()
    wt = nc.dram_tensor("weights", (D, D), f32, kind="ExternalInput").ap()
    out_rows = 4 * TILE_P if small_out else nc_nodes
    out = nc.dram_tensor("out", (out_rows, D), f32, kind="ExternalOutput").ap()

    with tile.TileContext(nc) as tc:
        with (
            tc.tile_pool(name="singles", bufs=1) as singles,
            tc.tile_pool(name="ns", bufs=b["ns"]) as nsp,
            tc.tile_pool(name="cum", bufs=b["cum"]) as cump,
            tc.tile_pool(name="work", bufs=b["work"]) as wp,
            tc.tile_pool(name="accp", bufs=b["accp"]) as accp,
            tc.tile_pool(name="prod", bufs=b["prod"]) as prodp,
            tc.tile_pool(name="outp", bufs=b["outp"]) as outp,
            tc.tile_pool(name="psum", bufs=b["psum"], space="PSUM") as pp,
        ):
            w_sb = singles.tile([D, D], f32)
            nc.sync.dma_start(out=w_sb, in_=wt)
            ident = singles.tile([TILE_P, TILE_P], f32)
            make_identity(nc, ident)
            # cum is produced and consumed back-to-back on DVE only, so a
            # single buffer (no rotation) is race-free; memset element 0 once.
            cum = singles.tile([TILE_P, NKEYS + 1, D], f32)
            nc.vector.memset(cum[:, 0, D - 1 : D], 0.0)

            rings = {"sp": nc.sync, "act": nc.scalar, "pool": nc.gpsimd}
            pending = []

            def emit_tail(st):
                ctxT_ps, inv, r0 = st
                if small_out:
                    r0 = (r0 // TILE_P) % 4 * TILE_P
                ctxT = wp.tile([TILE_P, TILE_P], f32, tag="ctxT_sb")
                if copy_eng == "pool":
                    nc.gpsimd.tensor_copy(out=ctxT, in_=ctxT_ps)
                elif copy_eng == "dve":
                    nc.vector.tensor_copy(out=ctxT, in_=ctxT_ps)
                else:
                    nc.scalar.copy(ctxT, ctxT_ps)
                out_ps = pp.tile([TILE_P, TILE_P], f32, tag="out_ps")
                nc.tensor.matmul(out_ps, lhsT=ctxT, rhs=w_sb, start=True, stop=True)
                ob = outp.tile([TILE_P, D], f32, tag="ob")
                nc.scalar.activation(ob, out_ps, Act.Relu, bias=0.0, scale=inv)
                rings[out_ring].dma_start(out=out[r0 : r0 + TILE_P, :], in_=ob)

            state = {}
            total = ntiles * repeat

            def s_load(t):
                r0 = (t % ntiles) * TILE_P
                ns = nsp.tile([TILE_P, NKEYS, D], f32, tag="ns")
                gr = gv_ring.split("/")[t % len(gv_ring.split("/"))]
                rings[gr].dma_start(
                    out=ns[:, 0:K, :], in_=gv[r0 : r0 + TILE_P, :, :]
                )
                rings[sv_ring].dma_start(out=ns[:, K, :], in_=sv[r0 : r0 + TILE_P, :])
                state[t] = {"ns": ns, "r0": r0}

            def s_scores(t):
                st = state[t]
                ns = st["ns"]
                selfv = ns[:, K, :]
                sv_b = selfv.unsqueeze(1).broadcast_to((TILE_P, NKEYS, D))
                nc.vector._custom_dve(
                    ops["DOT_SCAN_GAT"], out=cum[:, 1:, :], in0=ns, in1=sv_b
                )
                negscores = wp.tile([TILE_P, NKEYS], f32, tag="negscores")
                negmax = wp.tile([TILE_P, 1], f32, tag="negmax")
                nc.vector._custom_dve(
                    ops["DIFF_MIN_GAT"], out=negscores,
                    in0=cum[:, 0:NKEYS, D - 1], in1=cum[:, 1:, D - 1],
                    s0=1e38, accum_out=negmax,
                )
                e = wp.tile([TILE_P, NKEYS], f32, tag="e")
                sumexp = wp.tile([TILE_P, 1], f32, tag="sumexp")
                nc.scalar.activation(
                    e, negscores, Act.Exp, bias=negmax, scale=-1.0, accum_out=sumexp
                )
                st["e"] = e
                st["sumexp"] = sumexp

            def s_ctx(t):
                st = state[t]
                ns, e = st["ns"], st["e"]
                inv = wp.tile([TILE_P, 1], f32, tag="inv")
                nc.vector.reciprocal_approx_fast(inv, st["sumexp"])
                ctxT_ps = pp.tile([TILE_P, TILE_P], f32, tag="ctxT")
                first = [True]

                def tacc(src_, stop=False):
                    nc.tensor.matmul(
                        ctxT_ps, lhsT=src_, rhs=ident, is_transpose=True,
                        start=first[0], stop=stop,
                    )
                    first[0] = False

                kk = [0]
                done = [0]

                def emit_act():
                    for j in range(n_act):
                        k = kk[0]
                        pk = prodp.tile([TILE_P, D], f32, tag=f"pa{j}")
                        nc.scalar.mul(pk, ns[:, k, :], e[:, k : k + 1])
                        tacc(pk, stop=(done[0] + j == NKEYS - 1))
                        kk[0] += 1
                    done[0] += n_act

                def emit_pool():
                    # STT is illegal on Pool (walrus ISA check), so each
                    # product merges via its own PE transpose.
                    for j in range(n_pool):
                        k = kk[0]
                        pk = prodp.tile([TILE_P, D], f32, tag=f"pp{j}")
                        nc.gpsimd.tensor_scalar_mul(pk, ns[:, k, :], e[:, k : k + 1])
                        tacc(pk, stop=(done[0] + j == NKEYS - 1))
                        kk[0] += 1
                    done[0] += n_pool

                def emit_dve():
                    k = kk[0]
                    if dve_mode == "batch":
                        P = prodp.tile([TILE_P, n_dve, D], f32, tag="pdve")
                        e_b = e[:, k : k + n_dve].unsqueeze(2).broadcast_to(
                            (TILE_P, n_dve, D)
                        )
                        nc.vector.tensor_tensor(
                            out=P, in0=ns[:, k : k + n_dve, :], in1=e_b, op=Alu.mult
                        )
                        for j in range(n_dve):
                            tacc(P[:, j, :], stop=(done[0] + j == NKEYS - 1))
                        kk[0] += n_dve
                    else:
                        acc = accp.tile([TILE_P, D], f32, tag="acc")
                        nc.vector.tensor_scalar_mul(acc, ns[:, k, :], e[:, k : k + 1])
                        kk[0] += 1
                        for j in range(n_dve - 1):
                            k = kk[0]
                            acc2 = accp.tile([TILE_P, D], f32, tag="acc")
                            nc.vector.scalar_tensor_tensor(
                                out=acc2, in0=ns[:, k, :],
                                scalar=e[:, k : k + 1], in1=acc,
                                op0=Alu.mult, op1=Alu.add,
                            )
                            acc = acc2
                            kk[0] += 1
                        tacc(acc, stop=(done[0] + n_dve - 1 == NKEYS - 1))
                    done[0] += n_dve

                if pool_first:
                    emit_pool(); emit_dve(); emit_act()
                else:
                    emit_act(); emit_pool(); emit_dve()
                assert kk[0] == NKEYS and done[0] == NKEYS
                pending.append((ctxT_ps, inv, st["r0"]))

            if pipe:
                # depth = extra tiles of DMA lookahead beyond the minimal
                # 3-stage (load / scores / ctx) software pipeline.
                d = depth
                for i in range(total + 2 + d + 1):
                    if i < total:
                        s_load(i)
                    if ctx_first:
                        if d + 1 <= i < total + d + 1:
                            s_ctx(i - d - 1)
                        if d <= i < total + d:
                            s_scores(i - d)
                        if d + 1 <= i < total + d + 1:
                            if len(pending) > tail_defer:
                                emit_tail(pending.pop(0))
                            state.pop(i - d - 1)
                    else:
                        if d <= i < total + d:
                            s_scores(i - d)
                        if d + 1 <= i < total + d + 1:
                            s_ctx(i - d - 1)
                            if len(pending) > tail_defer:
                                emit_tail(pending.pop(0))
                            state.pop(i - d - 1)
            else:
                for t in range(total):
                    s_load(t)
                    s_scores(t)
                    s_ctx(t)
                    state.pop(t)
                    if len(pending) > 1:
                        emit_tail(pending.pop(0))
            while pending:
                emit_tail(pending.pop(0))

    nc.compile()
    return nc


def _get_nc():
    global _cached_nc
    if _cached_nc is None:
        _cached_nc = _build(**BEST)
    return _cached_nc


def run_sharded(self_vecs, neigh_vecs, weights, trace=False, nc=None):
    from concourse import bass_utils

    self_vecs = np.asarray(self_vecs, dtype=np.float32)
    neigh_vecs = np.asarray(neigh_vecs, dtype=np.float32)
    weights = np.asarray(weights, dtype=np.float32)

    n = self_vecs.shape[0]
    total = NCORES * NC_NODES
    pad = total - n
    if pad:
        self_p = np.concatenate([self_vecs, np.zeros((pad, D), np.float32)], axis=0)
        neigh_p = np.concatenate(
            [neigh_vecs, np.zeros((pad, K, D), np.float32)], axis=0
        )
    else:
        self_p, neigh_p = self_vecs, neigh_vecs

    in_maps = []
    for c in range(NCORES):
        lo, hi = c * NC_NODES, (c + 1) * NC_NODES
        in_maps.append(
            {
                "self_vecs": np.ascontiguousarray(self_p[lo:hi]),
                "neigh_vecs": np.ascontiguousarray(neigh_p[lo:hi]),
                "weights": weights,
            }
        )

    if nc is None:
        nc = _get_nc()
    try:
        res = bass_utils.run_bass_kernel_spmd(
            nc, in_maps, core_ids=list(range(NCORES)), trace=trace
        )
    except ModuleNotFoundError:
        import os

        os.environ["BASS_NEVER_TRACE"] = "1"
        res = bass_utils.run_bass_kernel_spmd(
            nc, in_maps, core_ids=list(range(NCORES)), trace=False
        )
    out = np.concatenate([res.results[c]["out"] for c in range(NCORES)], axis=0)[:n]
    return out, res


def kernel(self_vecs, neigh_vecs, weights):
    out, _ = run_sharded(self_vecs, neigh_vecs, weights, trace=False)
    return out
